# revision 4
# baseline (speedup 1.0000x reference)
"""CondGCN message-passing kernel for 8 Trainium2 NeuronCores.

Strategy:
  - Per-node transforms hoisted out of the per-edge loop: h_x=relu(x@Wx.T),
    h_ff=relu(x@Wff.T), h_bb=relu(x@Wbb.T), h_cx=relu(c@Wcx.T) computed once
    per node (1/8 of nodes per core, f32 matmuls, bias folded in via an
    augmented ones-row), stored as bf16 tables padded to 256B rows.
  - AllGather the per-core table slices -> full 250880-row table per core.
  - Edges sharded by DESTINATION range (no all-reduce needed). Host buckets
    edges by (super-pass, src-rank, dst-tile), pads each run to 128, and
    uploads int16 gather indices (wrapped 16-partition layout) + bf16
    local-dst values.
  - Per 128-edge chunk: dma_gather the source rows, build a one-hot
    [128e,128d] on DVE via is_equal against an iota, and accumulate
    aggT[64h,128d] += g.T @ onehot on the PE in PSUM. Self-loop added as one
    matmul vs the identity. Pool Linear applied in f32 per dst tile.
Outputs: out_x returned transposed per core ([64, 12544] f32) and assembled
on the host; out_c written row-major.
"""
import numpy as np
import ml_dtypes

import concourse.bass as bass
import concourse.bacc as bacc
import concourse.mybir as mybir
import concourse.tile as tile
from concourse.bass_utils import run_bass_kernel_spmd
from concourse._compat import cdiv

F32 = mybir.dt.float32
BF16 = mybir.dt.bfloat16
I16 = mybir.dt.int16
BF = ml_dtypes.bfloat16

NXN, NCN, E = 100000, 50000, 1000000
DH = 64
NCORE = 8
NXL, NCL = 12544, 6272          # per-core padded x/c rows
TX, TCC = NXL // 128, NCL // 128  # 98, 49 tiles per core
RANKROWS = 2 * NXL + NCL        # 31360 (h_ff | h_bb | h_cx)
HROWS = NCORE * RANKROWS        # 250880
SPT = 4                          # dst tiles per super-pass (PSUM banks)
NSP = cdiv(TX, SPT)             # 17
MAXIDX = 4096                    # max indices per dma_gather call

LAST_RESULTS = None


def _host_prep(inputs):
    x = np.asarray(inputs["x"], np.float32)
    c = np.asarray(inputs["c"], np.float32)
    xp = np.zeros((NCORE * NXL, DH), np.float32)
    xp[:NXN] = x
    cp = np.zeros((NCORE * NCL, DH), np.float32)
    cp[:NCN] = c

    def aug_tiles(arr, ntile):  # [n,64] -> [ntile,65,128] transposed + ones row
        n = ntile * 128
        t = arr[:n].reshape(ntile, 128, DH).transpose(0, 2, 1)  # [nt,64,128]
        out = np.empty((ntile, DH + 1, 128), np.float32)
        out[:, :DH] = t
        out[:, DH] = 1.0
        return out

    def waug(W, b):  # -> [65, 64] f32
        return np.vstack([np.asarray(W, np.float32).T,
                          np.asarray(b, np.float32)[None, :]])

    # per-edge streams: (dst, rank, local-gather-index)
    fs = np.asarray(inputs["e_fwd_src"], np.int64)
    fd = np.asarray(inputs["e_fwd_dst"], np.int64)
    bs = np.asarray(inputs["e_bwd_src"], np.int64)
    bd = np.asarray(inputs["e_bwd_dst"], np.int64)
    cs = np.asarray(inputs["e_cx_src"], np.int64)
    cd = np.asarray(inputs["e_cx_dst"], np.int64)
    dst = np.concatenate([fd, bs, cd])
    src_rank = np.concatenate([fs // NXL, bd // NXL, cs // NCL])
    gloc = np.concatenate([fs % NXL, NXL + bd % NXL, 2 * NXL + cs % NCL])

    core = dst // NXL
    dl = dst % NXL
    t_all = dl // 128        # dst tile within core 0..97
    loc = dl % 128
    sp_all = t_all // SPT

    # group id per edge: (core, sp, rank, tile)
    gid = ((core * NSP + sp_all) * NCORE + src_rank) * TX + t_all
    order = np.lexsort((gloc, gid))
    gid_s = gid[order]
    gloc_s = gloc[order]
    loc_s = loc[order]

    ngroups = NCORE * NSP * NCORE * TX
    cnt = np.bincount(gid_s, minlength=ngroups).reshape(NCORE, NSP, NCORE, TX)
    W = cdiv(cnt, 128) if isinstance(cnt, int) else -(-cnt // 128)
    Wmax = W.max(axis=0)  # [NSP, 8, TX] chunks per (sp, rank, tile) slot
    Wmax = np.maximum(Wmax, 1)
    # zero out slots for tiles not in their sp
    for sp in range(NSP):
        m = np.ones(TX, bool)
        m[sp * SPT:(sp + 1) * SPT] = False
        Wmax[sp, :, m] = 0
    slot_off = np.zeros(Wmax.size + 1, np.int64)
    np.cumsum(Wmax.ravel(), out=slot_off[1:])
    C_total = int(slot_off[-1])  # chunks per core (same all cores)

    # position of each edge within its group
    grp_start = np.zeros(ngroups + 1, np.int64)
    np.cumsum(np.bincount(gid_s, minlength=ngroups), out=grp_start[1:])
    within = np.arange(len(gid_s)) - grp_start[gid_s]
    slot_id = gid_s % (NSP * NCORE * TX)  # per-core slot index
    pos = slot_off[slot_id] * 128 + within  # chunk-stream position within core
    core_s = gid_s // (NSP * NCORE * TX)

    gl = np.zeros((NCORE, C_total * 128), np.int32)
    lc = np.full((NCORE, C_total * 128), 255, np.int32)
    flat = core_s * (C_total * 128) + pos
    gl.ravel()[flat] = gloc_s
    lc.ravel()[flat] = loc_s

    # build per-(sp, rank) call list: split spans into <=MAXIDX/128 chunk calls
    calls = []  # (sp, r, chunk_lo, chunk_hi) in global chunk coords
    for sp in range(NSP):
        for r in range(NCORE):
            base = (sp * NCORE + r) * TX
            lo = int(slot_off[base])
            hi = int(slot_off[base + TX])
            while lo < hi:
                n = min(hi - lo, MAXIDX // 128)
                calls.append((sp, r, lo, lo + n))
                lo += n

    # int16 wrapped index array: per call block [16, n*8] tiled to 128 rows
    idx_cols = sum((chi - clo) * 8 for _, _, clo, chi in calls)
    gidx16 = np.zeros((NCORE, 128, idx_cols), np.int16)
    col = 0
    call_cols = []
    for _, _, clo, chi in calls:
        n = (chi - clo) * 128
        blk = gl[:, clo * 128: chi * 128].reshape(NCORE, n // 16, 16)
        blk = blk.transpose(0, 2, 1).astype(np.int16)  # [core,16,n/16]
        gidx16[:, :, col: col + n // 16] = np.tile(blk, (1, 8, 1))
        call_cols.append(col)
        col += n // 16

    dstloc = lc.reshape(NCORE, C_total, 128).transpose(0, 2, 1)  # [core,128,C]
    dstloc = np.ascontiguousarray(dstloc.astype(np.float32))

    iota = np.broadcast_to(np.arange(128, dtype=BF), (128, 128)).copy()
    ident = np.eye(128, dtype=BF)

    in_maps = []
    for i in range(NCORE):
        in_maps.append(dict(
            xt=aug_tiles(xp[i * NXL:(i + 1) * NXL], TX),
            ct=aug_tiles(cp[i * NCL:(i + 1) * NCL], TCC),
            wff=waug(inputs["Wff"], inputs["bff"]),
            wbb=waug(inputs["Wbb"], inputs["bbb"]),
            wx=waug(inputs["Wx"], inputs["bx"]),
            wcx=waug(inputs["Wcx"], inputs["bcx"]),
            wc=waug(inputs["Wc"], inputs["bc"]),
            wpt=np.asarray(inputs["Wp"], np.float32).T.copy(),
            bp=np.asarray(inputs["bp"], np.float32).reshape(DH, 1),
            ident=ident, iota=iota,
            gidx=gidx16[i], dstloc=dstloc[i],
        ))
    meta = dict(C_total=C_total, calls=calls, call_cols=call_cols,
                Wmax=Wmax, slot_off=slot_off, idx_cols=idx_cols)
    return in_maps, meta


def _build_program(meta):
    C_total = meta["C_total"]
    calls = meta["calls"]
    call_cols = meta["call_cols"]
    Wmax = meta["Wmax"]
    slot_off = meta["slot_off"]
    idx_cols = meta["idx_cols"]

    nc = bacc.Bacc("TRN2", target_bir_lowering=False, debug=False,
                   num_devices=NCORE, num_swdge_queues=2)
    xt_t = nc.dram_tensor("xt", [TX, DH + 1, 128], F32, kind="ExternalInput")
    ct_t = nc.dram_tensor("ct", [TCC, DH + 1, 128], F32, kind="ExternalInput")
    w_t = {n: nc.dram_tensor(n, [DH + 1, DH], F32, kind="ExternalInput")
           for n in ("wff", "wbb", "wx", "wcx", "wc")}
    wpt_t = nc.dram_tensor("wpt", [DH, DH], F32, kind="ExternalInput")
    bp_t = nc.dram_tensor("bp", [DH, 1], F32, kind="ExternalInput")
    id_t = nc.dram_tensor("ident", [128, 128], BF16, kind="ExternalInput")
    iota_t = nc.dram_tensor("iota", [128, 128], BF16, kind="ExternalInput")
    gidx_t = nc.dram_tensor("gidx", [128, idx_cols], I16, kind="ExternalInput")
    dl_t = nc.dram_tensor("dstloc", [128, C_total], F32, kind="ExternalInput")
    oxt_t = nc.dram_tensor("out_xt", [DH, NXL], F32, kind="ExternalOutput")
    oc_t = nc.dram_tensor("out_c", [NCL, DH], F32, kind="ExternalOutput")
    hloc_t = nc.dram_tensor("hloc", [RANKROWS, 128], BF16)
    hfull_t = nc.dram_tensor("hfull", [HROWS, 128], BF16, addr_space="Shared")

    with tile.TileContext(nc) as tc:
        with (
            tc.tile_pool(name="const", bufs=1) as cpool,
            tc.tile_pool(name="hx", bufs=1) as hxpool,
            tc.tile_pool(name="p1", bufs=3) as p1,
            tc.tile_pool(name="ps1", bufs=2, space="PSUM") as ps1,
        ):
            w_s = {}
            for n in ("wff", "wbb", "wx", "wcx", "wc"):
                w_s[n] = cpool.tile([DH + 1, DH], F32, name=f"w_{n}")
                nc.sync.dma_start(out=w_s[n][:], in_=w_t[n][:])
            wpt_s = cpool.tile([DH, DH], F32)
            nc.sync.dma_start(out=wpt_s[:], in_=wpt_t[:])
            bp_s = cpool.tile([DH, 1], F32)
            nc.sync.dma_start(out=bp_s[:], in_=bp_t[:])
            id_s = cpool.tile([128, 128], BF16)
            nc.sync.dma_start(out=id_s[:], in_=id_t[:])
            iota_s = cpool.tile([128, 128], BF16)
            nc.sync.dma_start(out=iota_s[:], in_=iota_t[:])
            dl_s = cpool.tile([128, C_total], F32)
            nc.sync.dma_start(out=dl_s[:], in_=dl_t[:])

            hx_tiles = []
            # phase 1: x tiles -> h_ff, h_bb rows of hloc; h_x stays in SBUF
            for t in range(TX):
                lhs = p1.tile([DH + 1, 128], F32, tag="xt")
                nc.sync.dma_start(out=lhs[:], in_=xt_t[t])
                for kind, w in (("ff", "wff"), ("bb", "wbb"), ("x", "wx")):
                    ps = ps1.tile([128, DH], F32, tag="hps")
                    nc.tensor.matmul(out=ps[:], lhsT=lhs[:], rhs=w_s[w][:],
                                     start=True, stop=True)
                    if kind == "x":
                        hb = hxpool.tile([128, DH], BF16, tag=f"hx{t}", name=f"hxt{t}")
                    else:
                        hb = p1.tile([128, DH], BF16, tag="hb")
                    nc.scalar.activation(out=hb[:], in_=ps[:],
                                         func=mybir.ActivationFunctionType.Relu)
                    if kind == "ff":
                        nc.sync.dma_start(
                            out=hloc_t[t * 128:(t + 1) * 128, 0:DH], in_=hb[:])
                    elif kind == "bb":
                        nc.sync.dma_start(
                            out=hloc_t[NXL + t * 128: NXL + (t + 1) * 128, 0:DH],
                            in_=hb[:])
                    else:
                        hx_tiles.append(hb)
            # c tiles -> h_cx rows + out_c
            for t in range(TCC):
                lhs = p1.tile([DH + 1, 128], F32, tag="xt")
                nc.sync.dma_start(out=lhs[:], in_=ct_t[t])
                ps = ps1.tile([128, DH], F32, tag="hps")
                nc.tensor.matmul(out=ps[:], lhsT=lhs[:], rhs=w_s["wcx"][:],
                                 start=True, stop=True)
                hb = p1.tile([128, DH], BF16, tag="hb")
                nc.scalar.activation(out=hb[:], in_=ps[:],
                                     func=mybir.ActivationFunctionType.Relu)
                nc.sync.dma_start(
                    out=hloc_t[2 * NXL + t * 128: 2 * NXL + (t + 1) * 128, 0:DH],
                    in_=hb[:])
                ps2 = ps1.tile([128, DH], F32, tag="hps")
                nc.tensor.matmul(out=ps2[:], lhsT=lhs[:], rhs=w_s["wc"][:],
                                 start=True, stop=True)
                ob = p1.tile([128, DH], F32, tag="ob")
                nc.scalar.activation(out=ob[:], in_=ps2[:],
                                     func=mybir.ActivationFunctionType.Relu)
                nc.sync.dma_start(out=oc_t[t * 128:(t + 1) * 128, :], in_=ob[:])

            nc.gpsimd.collective_compute(
                "AllGather", mybir.AluOpType.bypass,
                replica_groups=[list(range(NCORE))],
                ins=[hloc_t[:]], outs=[hfull_t[:]],
            )

            with (
                tc.tile_pool(name="gb", bufs=1) as gbp,
                tc.tile_pool(name="stage", bufs=2) as stp,
                tc.tile_pool(name="oh", bufs=6) as ohp,
                tc.tile_pool(name="agg", bufs=1, space="PSUM") as aggp,
                tc.tile_pool(name="po", bufs=2, space="PSUM") as pop,
                tc.tile_pool(name="out2", bufs=3) as o2p,
            ):
                call_i = 0
                for sp in range(NSP):
                    t0, t1 = sp * SPT, min((sp + 1) * SPT, TX)
                    sp_calls = []
                    while call_i < len(calls) and calls[call_i][0] == sp:
                        sp_calls.append((call_i, *calls[call_i][1:]))
                        call_i += 1
                    gbufs = {}  # chunk range -> (tile, chunk_lo)
                    for ci, r, clo, chi in sp_calls:
                        nchunk = chi - clo
                        nidx = nchunk * 128
                        st = stp.tile([128, nidx // 16], I16, tag="st")
                        c0 = call_cols[ci]
                        nc.sync.dma_start(
                            out=st[:], in_=gidx_t[:, c0: c0 + nidx // 16])
                        gb = gbp.tile([128, nchunk, 128], BF16, tag=f"g{r}", name=f"gb_{ci}")
                        nc.gpsimd.dma_gather(
                            out_ap=gb[:], in_ap=hfull_t[r * RANKROWS:(r + 1) * RANKROWS, :],
                            idxs_ap=st[:], num_idxs=nidx, num_idxs_reg=nidx,
                            elem_size=128, single_packet=False, queue_num=r % 2,
                        )
                        gbufs[(clo, chi)] = gb
                    aggs = {}
                    started = set()
                    for t in range(t0, t1):
                        aggs[t] = aggp.tile([DH, 128], F32, tag=f"agg{t - t0}", name=f"agg_{t}")
                    for ci, r, clo, chi in sp_calls:
                        gb = gbufs[(clo, chi)]
                        for ch in range(clo, chi):
                            # which tile does this chunk belong to?
                            base = (sp * NCORE + r) * TX
                            t = int(np.searchsorted(slot_off[base: base + TX + 1],
                                                    ch, side="right")) - 1
                            oh = ohp.tile([128, 128], BF16, tag="oh")
                            nc.vector.tensor_scalar(
                                out=oh[:], in0=iota_s[:],
                                scalar1=dl_s[:, ch: ch + 1], scalar2=None,
                                op0=mybir.AluOpType.is_equal)
                            nc.tensor.matmul(
                                out=aggs[t][:], lhsT=gb[:, ch - clo, 0:DH],
                                rhs=oh[:], start=(t not in started), stop=False)
                            started.add(t)
                    for t in range(t0, t1):
                        nc.tensor.matmul(out=aggs[t][:], lhsT=hx_tiles[t][:],
                                         rhs=id_s[:], start=(t not in started),
                                         stop=True)
                        agg_sb = o2p.tile([DH, 128], F32, tag="aggsb")
                        nc.scalar.activation(
                            out=agg_sb[:], in_=aggs[t][:],
                            func=mybir.ActivationFunctionType.Copy)
                        po = pop.tile([DH, 128], F32, tag="po")
                        nc.tensor.matmul(out=po[:], lhsT=wpt_s[:],
                                         rhs=agg_sb[:], start=True, stop=True)
                        ob = o2p.tile([DH, 128], F32, tag="oxb")
                        nc.scalar.activation(
                            out=ob[:], in_=po[:],
                            func=mybir.ActivationFunctionType.Identity,
                            bias=bp_s[:])
                        nc.sync.dma_start(
                            out=oxt_t[:, t * 128:(t + 1) * 128], in_=ob[:])
    nc.compile()
    return nc


def kernel(**inputs):
    global LAST_RESULTS
    in_maps, meta = _host_prep(inputs)
    nc = _build_program(meta)
    res = run_bass_kernel_spmd(nc, in_maps, core_ids=list(range(NCORE)))
    LAST_RESULTS = res
    out_x = np.concatenate(
        [res.results[i]["out_xt"].T for i in range(NCORE)], axis=0)[:NXN]
    out_c = np.concatenate(
        [res.results[i]["out_c"] for i in range(NCORE)], axis=0)[:NCN]
    return np.ascontiguousarray(out_x), np.ascontiguousarray(out_c)


# revision 5
# speedup vs baseline: 1.0060x; 1.0060x over previous
"""CondGCN message-passing kernel for 8 Trainium2 NeuronCores.

Strategy:
  - Per-node transforms hoisted out of the per-edge loop: h_x=relu(x@Wx.T),
    h_ff=relu(x@Wff.T), h_bb=relu(x@Wbb.T), h_cx=relu(c@Wcx.T) computed once
    per node (1/8 of nodes per core, f32 matmuls, bias folded in via an
    augmented ones-row), stored as bf16 tables padded to 256B rows.
  - AllGather the per-core table slices -> full 250880-row table per core.
  - Edges sharded by DESTINATION range (no all-reduce needed). Host buckets
    edges by (super-pass, src-rank, dst-tile), pads each run to 128, and
    uploads int16 gather indices (wrapped 16-partition layout) + bf16
    local-dst values.
  - Per 128-edge chunk: dma_gather the source rows, build a one-hot
    [128e,128d] on DVE via is_equal against an iota, and accumulate
    aggT[64h,128d] += g.T @ onehot on the PE in PSUM. Self-loop added as one
    matmul vs the identity. Pool Linear applied in f32 per dst tile.
Outputs: out_x returned transposed per core ([64, 12544] f32) and assembled
on the host; out_c written row-major.
"""
import numpy as np
import ml_dtypes

import concourse.bass as bass
import concourse.bacc as bacc
import concourse.mybir as mybir
import concourse.tile as tile
from concourse.bass_utils import run_bass_kernel_spmd
from concourse._compat import cdiv

F32 = mybir.dt.float32
BF16 = mybir.dt.bfloat16
I16 = mybir.dt.int16
BF = ml_dtypes.bfloat16

NXN, NCN, E = 100000, 50000, 1000000
DH = 64
NCORE = 8
NXL, NCL = 12544, 6272          # per-core padded x/c rows
TX, TCC = NXL // 128, NCL // 128  # 98, 49 tiles per core
RANKROWS = 2 * NXL + NCL        # 31360 (h_ff | h_bb | h_cx)
HROWS = NCORE * RANKROWS        # 250880
SPT = 4                          # dst tiles per super-pass (PSUM banks)
NSP = cdiv(TX, SPT)             # 17
MAXIDX = 4096                    # max indices per dma_gather call

LAST_RESULTS = None


def _host_prep(inputs):
    x = np.asarray(inputs["x"], np.float32)
    c = np.asarray(inputs["c"], np.float32)
    xp = np.zeros((NCORE * NXL, DH), np.float32)
    xp[:NXN] = x
    cp = np.zeros((NCORE * NCL, DH), np.float32)
    cp[:NCN] = c

    def aug_tiles(arr, ntile):  # [n,64] -> [ntile,65,128] transposed + ones row
        n = ntile * 128
        t = arr[:n].reshape(ntile, 128, DH).transpose(0, 2, 1)  # [nt,64,128]
        out = np.empty((ntile, DH + 1, 128), np.float32)
        out[:, :DH] = t
        out[:, DH] = 1.0
        return out

    def waug(W, b):  # -> [65, 64] f32
        return np.vstack([np.asarray(W, np.float32).T,
                          np.asarray(b, np.float32)[None, :]])

    # per-edge streams: (dst, rank, local-gather-index)
    fs = np.asarray(inputs["e_fwd_src"], np.int64)
    fd = np.asarray(inputs["e_fwd_dst"], np.int64)
    bs = np.asarray(inputs["e_bwd_src"], np.int64)
    bd = np.asarray(inputs["e_bwd_dst"], np.int64)
    cs = np.asarray(inputs["e_cx_src"], np.int64)
    cd = np.asarray(inputs["e_cx_dst"], np.int64)
    dst = np.concatenate([fd, bs, cd])
    src_rank = np.concatenate([fs // NXL, bd // NXL, cs // NCL])
    gloc = np.concatenate([fs % NXL, NXL + bd % NXL, 2 * NXL + cs % NCL])

    core = dst // NXL
    dl = dst % NXL
    t_all = dl // 128        # dst tile within core 0..97
    loc = dl % 128
    sp_all = t_all // SPT

    # group id per edge: (core, sp, rank, tile)
    gid = ((core * NSP + sp_all) * NCORE + src_rank) * TX + t_all
    order = np.lexsort((gloc, gid))
    gid_s = gid[order]
    gloc_s = gloc[order]
    loc_s = loc[order]

    ngroups = NCORE * NSP * NCORE * TX
    cnt = np.bincount(gid_s, minlength=ngroups).reshape(NCORE, NSP, NCORE, TX)
    W = cdiv(cnt, 128) if isinstance(cnt, int) else -(-cnt // 128)
    Wmax = W.max(axis=0)  # [NSP, 8, TX] chunks per (sp, rank, tile) slot
    Wmax = np.maximum(Wmax, 1)
    # zero out slots for tiles not in their sp
    for sp in range(NSP):
        m = np.ones(TX, bool)
        m[sp * SPT:(sp + 1) * SPT] = False
        Wmax[sp, :, m] = 0
    slot_off = np.zeros(Wmax.size + 1, np.int64)
    np.cumsum(Wmax.ravel(), out=slot_off[1:])
    C_total = int(slot_off[-1])  # chunks per core (same all cores)

    # position of each edge within its group
    grp_start = np.zeros(ngroups + 1, np.int64)
    np.cumsum(np.bincount(gid_s, minlength=ngroups), out=grp_start[1:])
    within = np.arange(len(gid_s)) - grp_start[gid_s]
    slot_id = gid_s % (NSP * NCORE * TX)  # per-core slot index
    pos = slot_off[slot_id] * 128 + within  # chunk-stream position within core
    core_s = gid_s // (NSP * NCORE * TX)

    gl = np.zeros((NCORE, C_total * 128), np.int32)
    lc = np.full((NCORE, C_total * 128), 255, np.int32)
    flat = core_s * (C_total * 128) + pos
    gl.ravel()[flat] = gloc_s
    lc.ravel()[flat] = loc_s

    # build per-(sp, rank) call list: split spans into <=MAXIDX/128 chunk calls
    calls = []  # (sp, r, chunk_lo, chunk_hi) in global chunk coords
    for sp in range(NSP):
        for r in range(NCORE):
            base = (sp * NCORE + r) * TX
            lo = int(slot_off[base])
            hi = int(slot_off[base + TX])
            while lo < hi:
                n = min(hi - lo, MAXIDX // 128)
                calls.append((sp, r, lo, lo + n))
                lo += n

    # int16 wrapped index array: per call block [16, n*8] tiled to 128 rows
    idx_cols = sum((chi - clo) * 8 for _, _, clo, chi in calls)
    gidx16 = np.zeros((NCORE, 128, idx_cols), np.int16)
    col = 0
    call_cols = []
    for _, _, clo, chi in calls:
        n = (chi - clo) * 128
        blk = gl[:, clo * 128: chi * 128].reshape(NCORE, n // 16, 16)
        blk = blk.transpose(0, 2, 1).astype(np.int16)  # [core,16,n/16]
        gidx16[:, :, col: col + n // 16] = np.tile(blk, (1, 8, 1))
        call_cols.append(col)
        col += n // 16

    dstloc = lc.reshape(NCORE, C_total, 128).transpose(0, 2, 1)  # [core,128,C]
    dstloc = np.ascontiguousarray(dstloc.astype(np.float32))

    iota = np.broadcast_to(np.arange(128, dtype=BF), (128, 128)).copy()
    ident = np.eye(128, dtype=BF)

    in_maps = []
    for i in range(NCORE):
        in_maps.append(dict(
            xt=aug_tiles(xp[i * NXL:(i + 1) * NXL], TX),
            ct=aug_tiles(cp[i * NCL:(i + 1) * NCL], TCC),
            wff=waug(inputs["Wff"], inputs["bff"]),
            wbb=waug(inputs["Wbb"], inputs["bbb"]),
            wx=waug(inputs["Wx"], inputs["bx"]),
            wcx=waug(inputs["Wcx"], inputs["bcx"]),
            wc=waug(inputs["Wc"], inputs["bc"]),
            wpt=np.asarray(inputs["Wp"], np.float32).T.copy(),
            bp=np.asarray(inputs["bp"], np.float32).reshape(DH, 1),
            ident=ident, iota=iota,
            gidx=gidx16[i], dstloc=dstloc[i],
        ))
    meta = dict(C_total=C_total, calls=calls, call_cols=call_cols,
                Wmax=Wmax, slot_off=slot_off, idx_cols=idx_cols)
    return in_maps, meta


def _build_program(meta):
    C_total = meta["C_total"]
    calls = meta["calls"]
    call_cols = meta["call_cols"]
    Wmax = meta["Wmax"]
    slot_off = meta["slot_off"]
    idx_cols = meta["idx_cols"]

    nc = bacc.Bacc("TRN2", target_bir_lowering=False, debug=False,
                   num_devices=NCORE, num_swdge_queues=2)
    xt_t = nc.dram_tensor("xt", [TX, DH + 1, 128], F32, kind="ExternalInput")
    ct_t = nc.dram_tensor("ct", [TCC, DH + 1, 128], F32, kind="ExternalInput")
    w_t = {n: nc.dram_tensor(n, [DH + 1, DH], F32, kind="ExternalInput")
           for n in ("wff", "wbb", "wx", "wcx", "wc")}
    wpt_t = nc.dram_tensor("wpt", [DH, DH], F32, kind="ExternalInput")
    bp_t = nc.dram_tensor("bp", [DH, 1], F32, kind="ExternalInput")
    id_t = nc.dram_tensor("ident", [128, 128], BF16, kind="ExternalInput")
    iota_t = nc.dram_tensor("iota", [128, 128], BF16, kind="ExternalInput")
    gidx_t = nc.dram_tensor("gidx", [128, idx_cols], I16, kind="ExternalInput")
    dl_t = nc.dram_tensor("dstloc", [128, C_total], F32, kind="ExternalInput")
    oxt_t = nc.dram_tensor("out_xt", [DH, NXL], F32, kind="ExternalOutput")
    oc_t = nc.dram_tensor("out_c", [NCL, DH], F32, kind="ExternalOutput")
    hloc_t = nc.dram_tensor("hloc", [RANKROWS, 128], BF16)
    hfull_t = nc.dram_tensor("hfull", [HROWS, 128], BF16, addr_space="Shared")

    with tile.TileContext(nc) as tc:
        with (
            tc.tile_pool(name="const", bufs=1) as cpool,
            tc.tile_pool(name="hx", bufs=1) as hxpool,
            tc.tile_pool(name="p1", bufs=3) as p1,
            tc.tile_pool(name="ps1", bufs=2, space="PSUM") as ps1,
        ):
            w_s = {}
            for n in ("wff", "wbb", "wx", "wcx", "wc"):
                w_s[n] = cpool.tile([DH + 1, DH], F32, name=f"w_{n}")
                nc.sync.dma_start(out=w_s[n][:], in_=w_t[n][:])
            wpt_s = cpool.tile([DH, DH], F32)
            nc.sync.dma_start(out=wpt_s[:], in_=wpt_t[:])
            bp_s = cpool.tile([DH, 1], F32)
            nc.sync.dma_start(out=bp_s[:], in_=bp_t[:])
            id_s = cpool.tile([128, 128], BF16)
            nc.sync.dma_start(out=id_s[:], in_=id_t[:])
            iota_s = cpool.tile([128, 128], BF16)
            nc.sync.dma_start(out=iota_s[:], in_=iota_t[:])
            dl_s = cpool.tile([128, C_total], F32)
            nc.sync.dma_start(out=dl_s[:], in_=dl_t[:])

            hx_tiles = []
            # phase 1: x tiles -> h_ff, h_bb rows of hloc; h_x stays in SBUF
            for t in range(TX):
                lhs = p1.tile([DH + 1, 128], F32, tag="xt")
                nc.sync.dma_start(out=lhs[:], in_=xt_t[t])
                for kind, w in (("ff", "wff"), ("bb", "wbb"), ("x", "wx")):
                    ps = ps1.tile([128, DH], F32, tag="hps")
                    nc.tensor.matmul(out=ps[:], lhsT=lhs[:], rhs=w_s[w][:],
                                     start=True, stop=True)
                    if kind == "x":
                        hb = hxpool.tile([128, DH], BF16, tag=f"hx{t}", name=f"hxt{t}")
                    else:
                        hb = p1.tile([128, DH], BF16, tag="hb")
                    nc.scalar.activation(out=hb[:], in_=ps[:],
                                         func=mybir.ActivationFunctionType.Relu)
                    if kind == "ff":
                        nc.sync.dma_start(
                            out=hloc_t[t * 128:(t + 1) * 128, 0:DH], in_=hb[:])
                    elif kind == "bb":
                        nc.sync.dma_start(
                            out=hloc_t[NXL + t * 128: NXL + (t + 1) * 128, 0:DH],
                            in_=hb[:])
                    else:
                        hx_tiles.append(hb)
            # c tiles -> h_cx rows + out_c
            for t in range(TCC):
                lhs = p1.tile([DH + 1, 128], F32, tag="xt")
                nc.sync.dma_start(out=lhs[:], in_=ct_t[t])
                ps = ps1.tile([128, DH], F32, tag="hps")
                nc.tensor.matmul(out=ps[:], lhsT=lhs[:], rhs=w_s["wcx"][:],
                                 start=True, stop=True)
                hb = p1.tile([128, DH], BF16, tag="hb")
                nc.scalar.activation(out=hb[:], in_=ps[:],
                                     func=mybir.ActivationFunctionType.Relu)
                nc.sync.dma_start(
                    out=hloc_t[2 * NXL + t * 128: 2 * NXL + (t + 1) * 128, 0:DH],
                    in_=hb[:])
                ps2 = ps1.tile([128, DH], F32, tag="hps")
                nc.tensor.matmul(out=ps2[:], lhsT=lhs[:], rhs=w_s["wc"][:],
                                 start=True, stop=True)
                ob = p1.tile([128, DH], F32, tag="ob")
                nc.scalar.activation(out=ob[:], in_=ps2[:],
                                     func=mybir.ActivationFunctionType.Relu)
                nc.sync.dma_start(out=oc_t[t * 128:(t + 1) * 128, :], in_=ob[:])

            nc.gpsimd.collective_compute(
                "AllGather", mybir.AluOpType.bypass,
                replica_groups=[list(range(NCORE))],
                ins=[hloc_t[:]], outs=[hfull_t[:]],
            )

            with (
                tc.tile_pool(name="gb", bufs=2) as gbp,
                tc.tile_pool(name="stage", bufs=2) as stp,
                tc.tile_pool(name="oh", bufs=6) as ohp,
                tc.tile_pool(name="agg", bufs=1, space="PSUM") as aggp,
                tc.tile_pool(name="po", bufs=2, space="PSUM") as pop,
                tc.tile_pool(name="out2", bufs=3) as o2p,
            ):
                call_i = 0
                for sp in range(NSP):
                    t0, t1 = sp * SPT, min((sp + 1) * SPT, TX)
                    sp_calls = []
                    while call_i < len(calls) and calls[call_i][0] == sp:
                        sp_calls.append((call_i, *calls[call_i][1:]))
                        call_i += 1
                    gbufs = {}  # chunk range -> (tile, chunk_lo)
                    for ci, r, clo, chi in sp_calls:
                        nchunk = chi - clo
                        nidx = nchunk * 128
                        st = stp.tile([128, nidx // 16], I16, tag="st")
                        c0 = call_cols[ci]
                        nc.sync.dma_start(
                            out=st[:], in_=gidx_t[:, c0: c0 + nidx // 16])
                        gb = gbp.tile([128, nchunk, 128], BF16, tag=f"g{r}", name=f"gb_{ci}")
                        nc.gpsimd.dma_gather(
                            out_ap=gb[:], in_ap=hfull_t[r * RANKROWS:(r + 1) * RANKROWS, :],
                            idxs_ap=st[:], num_idxs=nidx, num_idxs_reg=nidx,
                            elem_size=128, single_packet=False, queue_num=r % 2,
                        )
                        gbufs[(clo, chi)] = gb
                    aggs = {}
                    started = set()
                    for t in range(t0, t1):
                        aggs[t] = aggp.tile([DH, 128], F32, tag=f"agg{t - t0}", name=f"agg_{t}")
                    for ci, r, clo, chi in sp_calls:
                        gb = gbufs[(clo, chi)]
                        for ch in range(clo, chi):
                            # which tile does this chunk belong to?
                            base = (sp * NCORE + r) * TX
                            t = int(np.searchsorted(slot_off[base: base + TX + 1],
                                                    ch, side="right")) - 1
                            oh = ohp.tile([128, 128], BF16, tag="oh")
                            nc.vector.tensor_scalar(
                                out=oh[:], in0=iota_s[:],
                                scalar1=dl_s[:, ch: ch + 1], scalar2=None,
                                op0=mybir.AluOpType.is_equal)
                            nc.tensor.matmul(
                                out=aggs[t][:], lhsT=gb[:, ch - clo, 0:DH],
                                rhs=oh[:], start=(t not in started), stop=False)
                            started.add(t)
                    for t in range(t0, t1):
                        nc.tensor.matmul(out=aggs[t][:], lhsT=hx_tiles[t][:],
                                         rhs=id_s[:], start=(t not in started),
                                         stop=True)
                        agg_sb = o2p.tile([DH, 128], F32, tag="aggsb")
                        nc.scalar.activation(
                            out=agg_sb[:], in_=aggs[t][:],
                            func=mybir.ActivationFunctionType.Copy)
                        po = pop.tile([DH, 128], F32, tag="po")
                        nc.tensor.matmul(out=po[:], lhsT=wpt_s[:],
                                         rhs=agg_sb[:], start=True, stop=True)
                        ob = o2p.tile([DH, 128], F32, tag="oxb")
                        nc.scalar.activation(
                            out=ob[:], in_=po[:],
                            func=mybir.ActivationFunctionType.Identity,
                            bias=bp_s[:])
                        nc.sync.dma_start(
                            out=oxt_t[:, t * 128:(t + 1) * 128], in_=ob[:])
    nc.compile()
    return nc


def kernel(**inputs):
    global LAST_RESULTS
    in_maps, meta = _host_prep(inputs)
    nc = _build_program(meta)
    res = run_bass_kernel_spmd(nc, in_maps, core_ids=list(range(NCORE)))
    LAST_RESULTS = res
    out_x = np.concatenate(
        [res.results[i]["out_xt"].T for i in range(NCORE)], axis=0)[:NXN]
    out_c = np.concatenate(
        [res.results[i]["out_c"] for i in range(NCORE)], axis=0)[:NCN]
    return np.ascontiguousarray(out_x), np.ascontiguousarray(out_c)
